# revision 16
# baseline (speedup 1.0000x reference)
"""KoLeo-loss kernel, 8 NeuronCores — hybrid AllGather + HBM streaming, v4.

v4 vs baseline:
  - AllGathers trigger ~12us in (vs ~36us): gpsimd's queue carries only
    [stream loads, agin DMAs, AG triggers, scatters]; identity/selector
    constants are host-fed instead of gpsimd-built.
  - Own queries live in 4 per-chunk tiles QTc[qc] so the agin reads of
    chunk 0/1 don't create whole-tile WAR stalls against chunk 2/3
    transposes.
  - PE transposes pair (v,t=0),(v,t=1) into one 2-bank PSUM tile so the
    PSUM->SBUF copies are 40x[128,256] instead of 80x[128,128].
  - Streamed x loads split across gpsimd+sync DMA queues.
  - No warm-up transposes.

(XBAR dma_start_transpose was tried and reverted: the tile framework
serializes DMA transposes against in-flight collectives, which blocks
the issuing engine's queue for the whole AG window. Shared-HBM agout
was tried and reverted: pair cores corrupt each other's RDH scratch.)

Groups g = 2q+h hold sub-chunk q of ranks 4h..4h+3 at band (r%4)*128.
q in {0,1} arrive by 2 chunked AllGathers; q in {2,3} streamed from HBM
and re-normalized locally. Diagonal of core r for query sub-chunk mc
lives in group 2*mc+(r>=4), column (r%4)*128+p: injected via host-fed
selector (lhsT) x band pattern (rhs) bf16 matmul.

Math: fp8e4 keys scaled 32/||x||, t-paired DoubleRow Gram = 1024*cos,
row max M -> loss_i = -0.5*ln(2 - M/512); host sums 8 partials.
"""

import sys
from contextlib import ExitStack

import numpy as np

sys.path.insert(0, "/opt/trn_rl_repo")

import concourse.mybir as mybir
import concourse.tile as tile
from concourse import bacc, bass_utils

F32 = mybir.dt.float32
BF16 = mybir.dt.bfloat16
F8 = mybir.dt.float8e4
AF = mybir.ActivationFunctionType
DR = mybir.MatmulPerfMode.DoubleRow

B, V, D = 4096, 2, 1024
NCORES = 8
MB = B // NCORES          # 512 own rows per core
NQ = MB // 128            # 4 own sub-chunks
NG = 8
T = 2
EPS = 1e-8
MASKV = -8192.0
AGQ = (0, 1)              # sub-chunks delivered by AllGather
STQ = (3, 2)              # streamed: q=3 lands early via sync, q=2 via gpsimd


def build():
    nc = bacc.Bacc("TRN2", debug=False, num_devices=NCORES)
    x_d = nc.dram_tensor("x", [B, V, D], F32, kind="ExternalInput").ap()
    xq_d = nc.dram_tensor("xq", [MB, V, D], F32, kind="ExternalInput").ap()
    band_d = nc.dram_tensor("bandpat", [128, 512], F32, kind="ExternalInput").ap()
    hsel_d = nc.dram_tensor("hseli", [128, 2, 128], F32, kind="ExternalInput").ap()
    out_d = nc.dram_tensor("out", [1, 1], F32, kind="ExternalOutput").ap()

    with ExitStack() as ctx:
        tc = ctx.enter_context(tile.TileContext(nc))
        const = ctx.enter_context(tc.tile_pool(name="const", bufs=1))
        xpool = ctx.enter_context(tc.tile_pool(name="xpool", bufs=8))
        xqpool = ctx.enter_context(tc.tile_pool(name="xqpool", bufs=4))
        ypool = ctx.enter_context(tc.tile_pool(name="ypool", bufs=3))
        sqpool = ctx.enter_context(tc.tile_pool(name="sqpool", bufs=2))
        sspool = ctx.enter_context(tc.tile_pool(name="sspool", bufs=3))
        accp = ctx.enter_context(tc.tile_pool(name="accp", bufs=4, space="PSUM"))
        trp = ctx.enter_context(tc.tile_pool(name="trp", bufs=3, space="PSUM"))
        smallp = ctx.enter_context(tc.tile_pool(name="smallp", bufs=1, space="PSUM"))
        dram = ctx.enter_context(tc.tile_pool(name="dram", bufs=1, space="DRAM"))

        # ---- constants (no gpsimd work: keep its queue free for AG) ----
        ones = const.tile([128, 1], F32, name="ones")
        nc.vector.memset(ones[:], 1.0)
        epsb = const.tile([128, 1], F32, name="epsb")
        nc.vector.memset(epsb[:], EPS)

        bandF = const.tile([128, 512], F32, name="bandF")
        nc.scalar.dma_start(bandF[:], band_d)
        bandB = const.tile([128, 512], BF16, name="bandB")
        nc.vector.tensor_copy(bandB[:], bandF[:])
        hsF = const.tile([128, 2, 128], F32, name="hsF")
        nc.scalar.dma_start(hsF[:], hsel_d)
        hselI = const.tile([128, 2, 128], BF16, name="hselI")
        nc.vector.tensor_copy(hselI[:], hsF[:])
        # identity built on gpsimd (first thing on its queue, ~1us;
        # host-fed identF lands too late behind the streamed-load pile)
        identF = const.tile([128, 128], F32, name="identF")
        nc.gpsimd.memset(identF[:], 0.0)
        nc.gpsimd.affine_select(
            out=identF[:], in_=identF[:], compare_op=mybir.AluOpType.not_equal,
            fill=1.0, base=0, pattern=[[-1, 128]], channel_multiplier=1)

        # ---- persistent buffers ----
        QTc = [const.tile([128, V, T, 128], F32, name=f"QT{qc}")
               for qc in range(NQ)]
        YTg = [const.tile([128, V, T, 512], F32, name=f"YT{g}")
               for g in range(NG)]
        mxs = const.tile([128, NG, V * NQ], F32, name="mxs")

        agin = [dram.tile([128, V, T, 128], F32, name=f"agin{q}") for q in AGQ]
        agout = [dram.tile([NCORES, 128, V, T, 128], F32, name=f"agout{q}")
                 for q in AGQ]

        def norm_quant(xt):
            """L2-normalize both views of a [128, V, D] f32 chunk, pack as
            fp8e4 scaled 32/||x|| into a [128, V, T, 128] f32-container tile."""
            ss = sspool.tile([128, V], F32, tag="ss", name="ss")
            sq = sqpool.tile([128, D], BF16, tag="sq", name="sq")
            for v in range(V):
                nc.scalar.activation(
                    sq[:], xt[:, v, :], AF.Square, accum_out=ss[:, v : v + 1])
            rec = sspool.tile([128, V], F32, tag="rec", name="rec")
            nc.vector.tensor_scalar_add(rec[:], ss[:], EPS)
            nc.vector.reciprocal(rec[:], rec[:])
            rs = sspool.tile([128, V], F32, tag="rs", name="rs")
            nc.scalar.activation(rs[:], rec[:], AF.Sqrt, scale=1024.0)
            ypk = ypool.tile([128, V, T, 128], F32, tag="ypk", name="ypk")
            yp8 = ypk.bitcast(F8)
            nc.vector.tensor_scalar_mul(
                yp8[:, 0].rearrange("p t k -> p (t k)"), xt[:, 0, :],
                rs[:, 0:1])
            nc.vector.tensor_scalar_mul(
                yp8[:, 1].rearrange("p t k -> p (t k)"), xt[:, 1, :],
                rs[:, 1:2])
            return ypk

        def transpose_into(ypk, dest_v_slices):
            """PE-transpose ypk's (v,t) f32-container tiles; copy per-v pairs
            from PSUM into dest_v_slices[v] (a [128, T, 128] f32 view)."""
            for v in range(V):
                tp2 = trp.tile([128, T, 128], F32, tag="tp", name="tp")
                for t in range(T):
                    nc.tensor.transpose(tp2[:, t], ypk[:, v, t], identF[:])
                nc.vector.tensor_copy(dest_v_slices[v], tp2[:])

        # ---- streamed x loads: q=2 on gpsimd (around AG), q=3 on sync ----
        st_tiles = {}

        def issue_stream_load(q, rr, eng):
            row0 = rr * MB + q * 128
            xt = xpool.tile([128, V, D], F32, tag="xraw", name="xraw")
            eng.dma_start(xt[:], x_d[row0 : row0 + 128])
            st_tiles[(q, rr)] = xt

        # own chunk loads on sync first (they gate the AG path); the two
        # AG-feeding chunks load per-view so the first Square can start
        # as soon as the v=0 plane lands.
        own_x = []
        for qc in range(NQ):
            xt = xqpool.tile([128, V, D], F32, tag="xown", name="xown")
            if qc in AGQ:
                for v in range(V):
                    nc.sync.dma_start(
                        xt[:, v : v + 1, :],
                        xq_d[128 * qc : 128 * (qc + 1), v : v + 1, :])
            else:
                nc.sync.dma_start(xt[:], xq_d[128 * qc : 128 * (qc + 1)])
            own_x.append(xt)

        # q=3 loads up front on sync (gpsimd's queue must stay empty so
        # the agin DMAs don't queue behind megabytes of loads; q=2 loads
        # are issued on gpsimd right after the AG triggers)
        for rr in range(NCORES):
            issue_stream_load(3, rr, nc.sync)

        # ---- own chunks -> QTc, AG for q in {0,1} ----
        for qc in range(NQ):
            ypk = norm_quant(own_x[qc])
            transpose_into(ypk, [QTc[qc][:, v] for v in range(V)])
            if qc in AGQ:
                nc.gpsimd.dma_start(agin[qc][:], QTc[qc][:])
                nc.gpsimd.collective_compute(
                    "AllGather", mybir.AluOpType.bypass,
                    replica_groups=[list(range(NCORES))],
                    ins=[agin[qc].opt()], outs=[agout[qc].opt()])

        # q=2 loads on gpsimd (its queue is free once both AGs are
        # triggered; data lands just in time for the q=2 consumption slot)
        for rr in range(NCORES):
            issue_stream_load(2, rr, nc.gpsimd)

        # ---- streamed chunks: local norm -> PE transpose into YTg ----
        for q in STQ:
            for rr in range(NCORES):
                ypk = norm_quant(st_tiles[(q, rr)])
                g = 2 * q + rr // 4
                c0 = 128 * (rr % 4)
                transpose_into(
                    ypk, [YTg[g][:, v, :, c0 : c0 + 128] for v in range(V)])

        # ---- AG scatter: agout -> YTg bands, split across two DMA
        # queues (gpsimd + scalar) so each group lands ~2x faster ----
        for q in AGQ:
            for rr in range(NCORES):
                c0 = 128 * (rr % 4)
                eng = nc.gpsimd if rr % 2 == 0 else nc.scalar
                eng.dma_start(
                    YTg[2 * q + rr // 4][:, :, :, c0 : c0 + 128], agout[q][rr])

        # ---- per-group Gram rows + row max ----
        Q8c = [
            QTc[qc].bitcast(F8)[:].rearrange("p v t (m b) -> p v b t m", b=4)
            for qc in range(NQ)
        ]
        for g in (6, 7, 4, 5, 0, 1, 2, 3):
            Y8r = YTg[g].bitcast(F8)[:].rearrange("p v t (k b) -> p v b t k", b=4)
            q_of_g, h_of_g = g // 2, g % 2
            for v in range(V):
                for mc in range(NQ):
                    has_mask = mc == q_of_g
                    acc = accp.tile([128, 512], F32, tag="acc", name="acc")
                    for b in range(4):
                        nc.tensor.matmul(
                            acc[:],
                            Q8c[mc][:, v, b],
                            Y8r[:, v, b],
                            start=(b == 0), stop=(b == 3 and not has_mask),
                            perf_mode=DR)
                    if has_mask:
                        nc.tensor.matmul(
                            acc[:], hselI[:, h_of_g], bandB[:],
                            start=False, stop=True, skip_group_check=True)
                    nc.vector.reduce_max(
                        mxs[:, g, v * NQ + mc : v * NQ + mc + 1], acc[:],
                        axis=mybir.AxisListType.X)

        # ---- finale ----
        fm = const.tile([128, V * NQ], F32, name="fm")
        nc.vector.reduce_max(
            fm[:], mxs.rearrange("p g c -> p c g"), axis=mybir.AxisListType.X)
        tt = const.tile([128, V * NQ], F32, name="tt")
        nc.vector.tensor_scalar(
            tt[:], fm[:], -1.0 / 512.0, 2.0, mybir.AluOpType.mult,
            mybir.AluOpType.add)
        lg = const.tile([128, V * NQ], F32, name="lg")
        nc.scalar.activation(lg[:], tt[:], AF.Ln, bias=epsb[:])
        ps2 = smallp.tile([1, V * NQ], F32, tag="sps", name="ps2")
        nc.tensor.matmul(ps2[:], ones[:], lg[:], start=True, stop=True)
        tot = const.tile([1, 1], F32, name="tot")
        nc.vector.reduce_sum(tot[:], ps2[:], axis=mybir.AxisListType.X)
        tots = const.tile([1, 1], F32, name="tots")
        nc.vector.tensor_scalar_mul(tots[:], tot[:], -0.5 / B)
        nc.sync.dma_start(out_d, tots[:])

    nc.compile()
    return nc


_CACHED = {}


def _run(x, trace=False):
    x = np.ascontiguousarray(np.asarray(x, dtype=np.float32))
    assert x.shape == (B, V, D), x.shape
    if "nc" not in _CACHED:
        _CACHED["nc"] = build()
    nc = _CACHED["nc"]
    in_maps = []
    for r in range(NCORES):
        band = np.zeros((128, 512), np.float32)
        col0 = (r % 4) * 128
        band[np.arange(128), col0 + np.arange(128)] = MASKV
        hseli = np.zeros((128, 2, 128), np.float32)
        hseli[np.arange(128), r // 4, np.arange(128)] = 1.0
        in_maps.append({
            "x": x,
            "xq": np.ascontiguousarray(x[MB * r : MB * (r + 1)]),
            "bandpat": band,
            "hseli": hseli,
        })
    res = bass_utils.run_bass_kernel_spmd(
        nc, in_maps, core_ids=list(range(NCORES)), trace=trace)
    partials = [np.float32(res.results[r]["out"][0, 0]) for r in range(NCORES)]
    total = np.float32(np.sum(np.array(partials, dtype=np.float32)))
    return total, res


def kernel(student_global_cls_tokens):
    total, _ = _run(student_global_cls_tokens, trace=False)
    return np.asarray(total, dtype=np.float32)


# revision 17
# speedup vs baseline: 1.7551x; 1.7551x over previous
"""KoLeo-loss kernel, 8 NeuronCores — hybrid AllGather + HBM streaming, v4.

v4 vs baseline:
  - AllGathers trigger ~12us in (vs ~36us): gpsimd's queue carries only
    [stream loads, agin DMAs, AG triggers, scatters]; identity/selector
    constants are host-fed instead of gpsimd-built.
  - Own queries live in 4 per-chunk tiles QTc[qc] so the agin reads of
    chunk 0/1 don't create whole-tile WAR stalls against chunk 2/3
    transposes.
  - PE transposes pair (v,t=0),(v,t=1) into one 2-bank PSUM tile so the
    PSUM->SBUF copies are 40x[128,256] instead of 80x[128,128].
  - Streamed x loads split across gpsimd+sync DMA queues.
  - No warm-up transposes.

(XBAR dma_start_transpose was tried and reverted: the tile framework
serializes DMA transposes against in-flight collectives, which blocks
the issuing engine's queue for the whole AG window. Shared-HBM agout
was tried and reverted: pair cores corrupt each other's RDH scratch.)

Groups g = 2q+h hold sub-chunk q of ranks 4h..4h+3 at band (r%4)*128.
q in {0,1} arrive by 2 chunked AllGathers; q in {2,3} streamed from HBM
and re-normalized locally. Diagonal of core r for query sub-chunk mc
lives in group 2*mc+(r>=4), column (r%4)*128+p: injected via host-fed
selector (lhsT) x band pattern (rhs) bf16 matmul.

Math: fp8e4 keys scaled 32/||x||, t-paired DoubleRow Gram = 1024*cos,
row max M -> loss_i = -0.5*ln(2 - M/512); host sums 8 partials.
"""

import sys
from contextlib import ExitStack

import numpy as np

sys.path.insert(0, "/opt/trn_rl_repo")

import concourse.mybir as mybir
import concourse.tile as tile
from concourse import bacc, bass_utils

F32 = mybir.dt.float32
BF16 = mybir.dt.bfloat16
F8 = mybir.dt.float8e4
AF = mybir.ActivationFunctionType
DR = mybir.MatmulPerfMode.DoubleRow

B, V, D = 4096, 2, 1024
NCORES = 8
MB = B // NCORES          # 512 own rows per core
NQ = MB // 128            # 4 own sub-chunks
NG = 8
T = 2
EPS = 1e-8
MASKV = -8192.0
AGQ = (0, 1)              # sub-chunks delivered by AllGather
STQ = (2, 3)              # sub-chunks streamed from HBM


def build():
    nc = bacc.Bacc("TRN2", debug=False, num_devices=NCORES)
    x_d = nc.dram_tensor("x", [B, V, D], F32, kind="ExternalInput").ap()
    xq_d = nc.dram_tensor("xq", [MB, V, D], F32, kind="ExternalInput").ap()
    band_d = nc.dram_tensor("bandpat", [128, 512], F32, kind="ExternalInput").ap()
    hsel_d = nc.dram_tensor("hseli", [128, 2, 128], F32, kind="ExternalInput").ap()
    out_d = nc.dram_tensor("out", [1, 1], F32, kind="ExternalOutput").ap()

    with ExitStack() as ctx:
        tc = ctx.enter_context(tile.TileContext(nc))
        const = ctx.enter_context(tc.tile_pool(name="const", bufs=1))
        xpool = ctx.enter_context(tc.tile_pool(name="xpool", bufs=8))
        xqpool = ctx.enter_context(tc.tile_pool(name="xqpool", bufs=4))
        ypool = ctx.enter_context(tc.tile_pool(name="ypool", bufs=3))
        sqpool = ctx.enter_context(tc.tile_pool(name="sqpool", bufs=2))
        sspool = ctx.enter_context(tc.tile_pool(name="sspool", bufs=3))
        accp = ctx.enter_context(tc.tile_pool(name="accp", bufs=4, space="PSUM"))
        trp = ctx.enter_context(tc.tile_pool(name="trp", bufs=3, space="PSUM"))
        smallp = ctx.enter_context(tc.tile_pool(name="smallp", bufs=1, space="PSUM"))
        dram = ctx.enter_context(tc.tile_pool(name="dram", bufs=1, space="DRAM"))

        # ---- constants (no gpsimd work: keep its queue free for AG) ----
        ones = const.tile([128, 1], F32, name="ones")
        nc.vector.memset(ones[:], 1.0)
        epsb = const.tile([128, 1], F32, name="epsb")
        nc.vector.memset(epsb[:], EPS)

        bandF = const.tile([128, 512], F32, name="bandF")
        nc.scalar.dma_start(bandF[:], band_d)
        bandB = const.tile([128, 512], BF16, name="bandB")
        nc.vector.tensor_copy(bandB[:], bandF[:])
        hsF = const.tile([128, 2, 128], F32, name="hsF")
        nc.scalar.dma_start(hsF[:], hsel_d)
        hselI = const.tile([128, 2, 128], BF16, name="hselI")
        nc.vector.tensor_copy(hselI[:], hsF[:])
        # identity built on gpsimd (first thing on its queue, ~1us;
        # host-fed identF lands too late behind the streamed-load pile)
        identF = const.tile([128, 128], F32, name="identF")
        nc.gpsimd.memset(identF[:], 0.0)
        nc.gpsimd.affine_select(
            out=identF[:], in_=identF[:], compare_op=mybir.AluOpType.not_equal,
            fill=1.0, base=0, pattern=[[-1, 128]], channel_multiplier=1)

        # ---- persistent buffers ----
        QTc = [const.tile([128, V, T, 128], F32, name=f"QT{qc}")
               for qc in range(NQ)]
        YTg = [const.tile([128, V, T, 512], F32, name=f"YT{g}")
               for g in range(NG)]
        mxs = const.tile([128, NG, V * NQ], F32, name="mxs")

        agin = [dram.tile([128, V, T, 128], F32, name=f"agin{q}") for q in AGQ]
        agout = [dram.tile([NCORES, 128, V, T, 128], F32, name=f"agout{q}")
                 for q in AGQ]

        def norm_quant(xt):
            """L2-normalize both views of a [128, V, D] f32 chunk, pack as
            fp8e4 scaled 32/||x|| into a [128, V, T, 128] f32-container tile."""
            ss = sspool.tile([128, V], F32, tag="ss", name="ss")
            sq = sqpool.tile([128, D], BF16, tag="sq", name="sq")
            for v in range(V):
                nc.scalar.activation(
                    sq[:], xt[:, v, :], AF.Square, accum_out=ss[:, v : v + 1])
            rec = sspool.tile([128, V], F32, tag="rec", name="rec")
            nc.vector.tensor_scalar_add(rec[:], ss[:], EPS)
            nc.vector.reciprocal(rec[:], rec[:])
            rs = sspool.tile([128, V], F32, tag="rs", name="rs")
            nc.scalar.activation(rs[:], rec[:], AF.Sqrt, scale=1024.0)
            ypk = ypool.tile([128, V, T, 128], F32, tag="ypk", name="ypk")
            yp8 = ypk.bitcast(F8)
            nc.vector.tensor_scalar_mul(
                yp8[:, 0].rearrange("p t k -> p (t k)"), xt[:, 0, :],
                rs[:, 0:1])
            nc.vector.tensor_scalar_mul(
                yp8[:, 1].rearrange("p t k -> p (t k)"), xt[:, 1, :],
                rs[:, 1:2])
            return ypk

        def transpose_into(ypk, dest_v_slices):
            """PE-transpose ypk's (v,t) f32-container tiles; copy per-v pairs
            from PSUM into dest_v_slices[v] (a [128, T, 128] f32 view)."""
            for v in range(V):
                tp2 = trp.tile([128, T, 128], F32, tag="tp", name="tp")
                for t in range(T):
                    nc.tensor.transpose(tp2[:, t], ypk[:, v, t], identF[:])
                nc.vector.tensor_copy(dest_v_slices[v], tp2[:])

        # ---- streamed x loads: q=2 on gpsimd (around AG), q=3 on sync ----
        st_tiles = {}

        def issue_stream_load(q, rr, eng):
            row0 = rr * MB + q * 128
            xt = xpool.tile([128, V, D], F32, tag="xraw", name="xraw")
            eng.dma_start(xt[:], x_d[row0 : row0 + 128])
            st_tiles[(q, rr)] = xt

        # own chunk loads on sync first (they gate the AG path); the two
        # AG-feeding chunks load per-view so the first Square can start
        # as soon as the v=0 plane lands.
        own_x = []
        for qc in range(NQ):
            xt = xqpool.tile([128, V, D], F32, tag="xown", name="xown")
            if qc in AGQ:
                for v in range(V):
                    nc.sync.dma_start(
                        xt[:, v : v + 1, :],
                        xq_d[128 * qc : 128 * (qc + 1), v : v + 1, :])
            else:
                nc.sync.dma_start(xt[:], xq_d[128 * qc : 128 * (qc + 1)])
            own_x.append(xt)

        # streamed loads up front on two queues: q=2 on gpsimd, q=3 on
        # sync (the agin DMAs queue behind gpsimd's 8MB but still trigger
        # well before the ~50us barrier ends)
        for rr in range(NCORES):
            issue_stream_load(2, rr, nc.gpsimd)
        for rr in range(NCORES):
            issue_stream_load(3, rr, nc.sync)

        # ---- own chunks -> QTc, AG for q in {0,1} ----
        for qc in range(NQ):
            ypk = norm_quant(own_x[qc])
            transpose_into(ypk, [QTc[qc][:, v] for v in range(V)])
            if qc in AGQ:
                nc.gpsimd.dma_start(agin[qc][:], QTc[qc][:])
                nc.gpsimd.collective_compute(
                    "AllGather", mybir.AluOpType.bypass,
                    replica_groups=[list(range(NCORES))],
                    ins=[agin[qc].opt()], outs=[agout[qc].opt()])

        # ---- streamed chunks: local norm -> PE transpose into YTg ----
        for q in STQ:
            for rr in range(NCORES):
                ypk = norm_quant(st_tiles[(q, rr)])
                g = 2 * q + rr // 4
                c0 = 128 * (rr % 4)
                transpose_into(
                    ypk, [YTg[g][:, v, :, c0 : c0 + 128] for v in range(V)])

        # ---- AG scatter: agout -> YTg bands. gpsimd ONLY: any engine
        # whose FIFO holds an AG-completion wait must have no later
        # time-critical work (a scalar-queue scatter once stalled the
        # whole normalize pipeline behind an in-flight collective) ----
        for q in AGQ:
            for rr in range(NCORES):
                c0 = 128 * (rr % 4)
                nc.gpsimd.dma_start(
                    YTg[2 * q + rr // 4][:, :, :, c0 : c0 + 128], agout[q][rr])

        # ---- per-group Gram rows + row max ----
        Q8c = [
            QTc[qc].bitcast(F8)[:].rearrange("p v t (m b) -> p v b t m", b=4)
            for qc in range(NQ)
        ]
        for g in (4, 5, 6, 7, 0, 1, 2, 3):
            Y8r = YTg[g].bitcast(F8)[:].rearrange("p v t (k b) -> p v b t k", b=4)
            q_of_g, h_of_g = g // 2, g % 2
            for v in range(V):
                for mc in range(NQ):
                    has_mask = mc == q_of_g
                    acc = accp.tile([128, 512], F32, tag="acc", name="acc")
                    for b in range(4):
                        nc.tensor.matmul(
                            acc[:],
                            Q8c[mc][:, v, b],
                            Y8r[:, v, b],
                            start=(b == 0), stop=(b == 3 and not has_mask),
                            perf_mode=DR)
                    if has_mask:
                        nc.tensor.matmul(
                            acc[:], hselI[:, h_of_g], bandB[:],
                            start=False, stop=True, skip_group_check=True)
                    nc.vector.reduce_max(
                        mxs[:, g, v * NQ + mc : v * NQ + mc + 1], acc[:],
                        axis=mybir.AxisListType.X)

        # ---- finale ----
        fm = const.tile([128, V * NQ], F32, name="fm")
        nc.vector.reduce_max(
            fm[:], mxs.rearrange("p g c -> p c g"), axis=mybir.AxisListType.X)
        tt = const.tile([128, V * NQ], F32, name="tt")
        nc.vector.tensor_scalar(
            tt[:], fm[:], -1.0 / 512.0, 2.0, mybir.AluOpType.mult,
            mybir.AluOpType.add)
        lg = const.tile([128, V * NQ], F32, name="lg")
        nc.scalar.activation(lg[:], tt[:], AF.Ln, bias=epsb[:])
        ps2 = smallp.tile([1, V * NQ], F32, tag="sps", name="ps2")
        nc.tensor.matmul(ps2[:], ones[:], lg[:], start=True, stop=True)
        tot = const.tile([1, 1], F32, name="tot")
        nc.vector.reduce_sum(tot[:], ps2[:], axis=mybir.AxisListType.X)
        tots = const.tile([1, 1], F32, name="tots")
        nc.vector.tensor_scalar_mul(tots[:], tot[:], -0.5 / B)
        nc.sync.dma_start(out_d, tots[:])

    nc.compile()
    return nc


_CACHED = {}


def _run(x, trace=False):
    x = np.ascontiguousarray(np.asarray(x, dtype=np.float32))
    assert x.shape == (B, V, D), x.shape
    if "nc" not in _CACHED:
        _CACHED["nc"] = build()
    nc = _CACHED["nc"]
    in_maps = []
    for r in range(NCORES):
        band = np.zeros((128, 512), np.float32)
        col0 = (r % 4) * 128
        band[np.arange(128), col0 + np.arange(128)] = MASKV
        hseli = np.zeros((128, 2, 128), np.float32)
        hseli[np.arange(128), r // 4, np.arange(128)] = 1.0
        in_maps.append({
            "x": x,
            "xq": np.ascontiguousarray(x[MB * r : MB * (r + 1)]),
            "bandpat": band,
            "hseli": hseli,
        })
    res = bass_utils.run_bass_kernel_spmd(
        nc, in_maps, core_ids=list(range(NCORES)), trace=trace)
    partials = [np.float32(res.results[r]["out"][0, 0]) for r in range(NCORES)]
    total = np.float32(np.sum(np.array(partials, dtype=np.float32)))
    return total, res


def kernel(student_global_cls_tokens):
    total, _ = _run(student_global_cls_tokens, trace=False)
    return np.asarray(total, dtype=np.float32)
